# revision 6
# baseline (speedup 1.0000x reference)
"""Trainium2 Bass kernel for nn_AttentionMLP (B=4, S=4096, two attention+MLP stages).

Sharding: 8 cores = 4 batches x 2 sequence-halves. Each core computes its
2048 query rows end-to-end; pairwise AllGathers (chunked, pipelined)
exchange the stage-1 output halves so stage 2 attends over the full
sequence.

v2 scheduling (vs baseline), numerics unchanged (fp32r everywhere):
  - Scores are K=64 matmuls (half the PE rows). q/k projection weights are
    widened to [w|w] so qT/kT land duplicated across both partition halves;
    score matmuls for even/odd key blocks issue to row-groups (0,0)/(64,0)
    via tile_position and execute concurrently: ~2x score throughput.
  - Software pipelining: group g+1's score matmuls are emitted before group
    g's attn@v so the PE queue never head-of-line blocks on the exp; each
    chunk's MLP is emitted inside the next chunk's group loop. ACT (exp) is
    the bottleneck engine and stays saturated.
  - Output biases moved off the PE (no K=1 ones matmuls): stage-1 bias via
    tensor_scalar ADD on the outT drain; stage-2 via tensor_add with a
    replicated bias tile (separate f32 bias input tensor).
"""

import numpy as np
from contextlib import ExitStack

import concourse.bass as bass
import concourse.tile as tile
from concourse import bacc, mybir
from concourse import bass_utils

F32 = mybir.dt.float32
F32R = mybir.dt.float32r
EXP = mybir.ActivationFunctionType.Exp
ADD = mybir.AluOpType.add
MIN = mybir.AluOpType.min
MAX = mybir.AluOpType.max

N_CORES = 8
B, S, D = 4, 4096, 64
R = S // 2            # own query rows per core
HD = 256
NCK = R // 512        # si-chunks per core (4 x 512)
NJB = S // 128        # key blocks (32 x 128)
# exp-group sizes per chunk: one double-buffered [128, 1536] scores tag
# (6 PSUM banks) + av (1) + mlp/proj (1) = 8 banks.
GROUPS = [3] * 10 + [2]
assert sum(GROUPS) == NJB

# f32r weight pack (col offsets in f32 words)
# region A (partitions 0-63, one 576-col block per stage): wq_dup|wk_dup|wv|w1t
WQ, WK, WV, W1T = 0, 128, 256, 320
RA = 576
W2T0 = 2 * RA                    # region B: [128, 128] per stage
WB = W2T0 + 256
# f32 bias pack
B1C0 = 0                         # [128, 2] per stage -> cols 0..3
B2C1 = 4                         # [64, 1] stage-1 b2_eff (per-partition)
B2R2 = 5                         # [128, 64] stage-2 b2_eff (replicated)
BF32C = B2R2 + 64


def build_nc(n_cores=N_CORES, reps=1, exch_chunks=NCK):
    nc = bacc.Bacc("TRN2", target_bir_lowering=False, debug=False,
                   num_devices=n_cores)

    xT_d = nc.dram_tensor("xT", [64, S], F32R, kind="ExternalInput").ap()
    w_d = nc.dram_tensor("wpack", [128, WB], F32R, kind="ExternalInput").ap()
    b_d = nc.dram_tensor("bias32", [128, BF32C], F32,
                         kind="ExternalInput").ap()
    out_d = nc.dram_tensor("out1", [R, 64], F32, kind="ExternalOutput").ap()

    with tile.TileContext(nc) as tc, ExitStack() as ctx:
        consts = ctx.enter_context(tc.tile_pool(name="consts", bufs=1))
        sb = ctx.enter_context(tc.tile_pool(name="sb", bufs=1))
        ps = ctx.enter_context(tc.tile_pool(name="ps", bufs=2, space="PSUM"))
        dram = ctx.enter_context(tc.tile_pool(name="dram", bufs=1,
                                              space="DRAM"))

        wt = consts.tile([128, WB], F32R)
        nc.sync.dma_start(wt[:, 0:RA], w_d[:, 0:RA])
        nc.scalar.dma_start(wt[:, RA:WB], w_d[:, RA:WB])
        bt = consts.tile([128, BF32C], F32)
        nc.gpsimd.dma_start(bt[:], b_d[:])
        dma_engines = [nc.sync, nc.scalar, nc.gpsimd]

        for _rep in range(reps):
            _body(nc, sb, ps, dram, wt, bt, dma_engines,
                  xT_d, out_d, _rep, n_cores)

    nc.compile()
    return nc


def _body(nc, sb, ps, dram, wt, bt, dma_engines, xT_d, out_d, rep, n_cores):
    xT = sb.tile([64, S], F32R, tag="xt", bufs=2, name=f"xT_{rep}")
    for n in range(8):
        dma_engines[n % 3].dma_start(xT[:, n * 512:(n + 1) * 512],
                                     xT_d[:, n * 512:(n + 1) * 512])
    outT = sb.tile([64, R], F32R, tag="outT", bufs=2, name=f"outT_{rep}")
    xT2 = sb.tile([64, S], F32R, tag="xt", bufs=2, name=f"xT2_{rep}")

    def alloc_proj(sfx):
        # qT/kT duplicated across partition halves (row-packed score tiles)
        qT = sb.tile([128, R], F32R, tag=f"qT{sfx}", name=f"qT{sfx}_{rep}")
        kT = sb.tile([128, S], F32R, tag=f"kT{sfx}", name=f"kT{sfx}_{rep}")
        vA = sb.tile([128, NJB, 65], F32R, tag=f"vA{sfx}",
                     name=f"vA{sfx}_{rep}")
        onescol = sb.tile([128, NJB], F32, tag="onescol", bufs=2,
                          name=f"ones{sfx}_{rep}")
        nc.vector.memset(onescol[:], 1.0)
        nc.vector.tensor_copy(vA[:, :, 64:65], onescol[:].unsqueeze(2))
        return qT, kT, vA

    qT1, kT1, vA1 = alloc_proj(0)
    qT2, kT2, vA2 = alloc_proj(1)

    # --- projection emitters ------------------------------------------------
    def emit_k(sfx, kT, src, sl):
        wsl = wt[0:64, sfx * RA:(sfx + 1) * RA]
        pk = ps.tile([128, 512], F32, tag="mlp", bufs=1)
        nc.tensor.matmul(pk[:], wsl[:, WK:WK + 128],
                         src[:, sl * 512:(sl + 1) * 512],
                         start=True, stop=True)
        nc.vector.tensor_copy(kT[:, sl * 512:(sl + 1) * 512], pk[:])

    def emit_q(sfx, qT, src, sl):
        wsl = wt[0:64, sfx * RA:(sfx + 1) * RA]
        pq = ps.tile([128, 512], F32, tag="mlp", bufs=1)
        nc.tensor.matmul(pq[:], wsl[:, WQ:WQ + 128],
                         src[:, sl * 512:(sl + 1) * 512],
                         start=True, stop=True)
        nc.vector.tensor_copy(qT[:, sl * 512:(sl + 1) * 512], pq[:])

    def emit_v(sfx, vA, src, sl, jb0):
        wsl = wt[0:64, sfx * RA:(sfx + 1) * RA]
        pv = ps.tile([128, 4, 64], F32, tag="mlp", bufs=1)
        for b in range(4):
            nc.tensor.matmul(pv[:, b, :],
                             src[:, sl * 512 + b * 128:sl * 512 + (b + 1) * 128],
                             wsl[:, WV:WV + 64], start=True, stop=True)
        nc.vector.tensor_copy(vA[:, jb0:jb0 + 4, 0:64], pv[:])

    def proj_unit(sfx, qT, kT, vA, src, sl, with_q=True):
        def fn():
            emit_k(sfx, kT, src, sl)
            emit_v(sfx, vA, src, sl, 4 * sl)
            if with_q and sl < NCK:
                emit_q(sfx, qT, src, sl)
        return fn

    # --- stage-1 -> stage-2 exchange ---------------------------------------
    bounce_ins = [dram.tile([64, 512], F32R, name=f"bi_{rep}_{n}",
                            tag=f"bi{n}") for n in range(NCK)]
    bounce_outs = [dram.tile([2, 64, 512], F32R, name=f"bo_{rep}_{n}",
                             tag=f"bo{n}") for n in range(NCK)]

    def exchange(n):
        nc.sync.dma_start(bounce_ins[n][:], outT[:, n * 512:(n + 1) * 512])
        if n_cores > 1:
            nc.gpsimd.collective_compute(
                "AllGather", mybir.AluOpType.bypass,
                replica_groups=[[0, 1], [2, 3], [4, 5], [6, 7]],
                ins=[bounce_ins[n][:].opt()],
                outs=[bounce_outs[n][:].opt()])
        else:
            for m in range(2):
                nc.sync.dma_start(bounce_outs[n][m], bounce_ins[n][:])
        for m in range(2):
            dma_engines[(m * NCK + n) % 3].dma_start(
                xT2[:, m * R + n * 512:m * R + (n + 1) * 512],
                bounce_outs[n][m])

    # --- one attention+MLP stage -------------------------------------------
    def stage(sfx, qT, kT, vA, group_emits, write_out, after_mlp):
        wsl = wt[0:64, sfx * RA:(sfx + 1) * RA]
        w2t = wt[:, W2T0 + sfx * 128:W2T0 + (sfx + 1) * 128]
        aTs = [None] * NCK

        def mlp(n):
            aT = aTs[n]
            u = sb.tile([128, 1024], F32, tag="u", bufs=2)
            r = sb.tile([128, 1024], F32, tag="r", bufs=2)
            for j in range(2):
                ph = ps.tile([128, 512], F32, tag="mlp", bufs=1)
                nc.tensor.matmul(ph[:],
                                 wsl[:, W1T + j * 128:W1T + (j + 1) * 128],
                                 aT[:], start=True, stop=True)
                b1j = bt[:, sfx * 2 + j:sfx * 2 + j + 1]
                nc.vector.tensor_scalar(u[:, j * 512:(j + 1) * 512], ph[:],
                                        b1j, 0.0, op0=ADD, op1=MIN)
                nc.vector.tensor_scalar(r[:, j * 512:(j + 1) * 512], ph[:],
                                        b1j, 0.0, op0=ADD, op1=MAX)
            e = sb.tile([128, 1024], F32, tag="e", bufs=2)
            nc.scalar.activation(e[:], u[:], EXP)
            hT = sb.tile([128, 1024], F32R, tag="hT", bufs=2)
            nc.vector.tensor_add(hT[:], r[:], e[:])
            write_out(n, hT, w2t)
            if after_mlp is not None:
                after_mlp(n)

        for n in range(NCK):
            av_box = [None]

            def emit_av(ex, jb0, gsz):
                if av_box[0] is None:
                    av_box[0] = ps.tile([65, 512], F32, tag="av", bufs=1,
                                        name=f"av_{rep}_{sfx}_{n}")
                for i in range(gsz):
                    nc.tensor.matmul(av_box[0][:], vA[:, jb0 + i, :],
                                     ex[:, i * 512:(i + 1) * 512],
                                     start=(jb0 + i == 0),
                                     stop=(jb0 + i == NJB - 1))

            pend = None
            jb = 0
            for gi, gsz in enumerate(GROUPS):
                for fn in group_emits.get((n, gi), ()):
                    fn()
                st = ps.tile([128, gsz * 512], F32, tag="sA", bufs=2)
                for i in range(gsz):
                    h = (jb + i) % 2
                    nc.tensor.matmul(
                        st[:, i * 512:(i + 1) * 512],
                        kT[h * 64:(h + 1) * 64,
                           (jb + i) * 128:(jb + i + 1) * 128],
                        qT[h * 64:(h + 1) * 64, n * 512:(n + 1) * 512],
                        start=True, stop=True, tile_position=(h * 64, 0))
                ex = sb.tile([128, gsz * 512], F32R, tag="exp", bufs=4)
                nc.scalar.activation(ex[:], st[:], EXP)
                if gi == 2 and n > 0:
                    mlp(n - 1)
                if pend is not None:
                    emit_av(*pend)
                pend = (ex, jb, gsz)
                jb += gsz
            emit_av(*pend)
            av = av_box[0]

            # normalize: aT = av[0:64] / av[64]; immediate copies drain the
            # av PSUM bank for the next chunk.
            rs = sb.tile([1, 512], F32, tag="rs", bufs=3)
            nc.vector.tensor_copy(rs[:], av[64:65, :])
            araw = sb.tile([64, 512], F32, tag="araw", bufs=3)
            nc.vector.tensor_copy(araw[:], av[0:64, :])
            rr = sb.tile([1, 512], F32, tag="rr", bufs=3)
            nc.vector.reciprocal_approx_fast(rr[:], rs[:])
            rb = sb.tile([64, 512], F32, tag="rb", bufs=3)
            nc.gpsimd.partition_broadcast(rb[:], rr[:])
            aT = sb.tile([64, 512], F32R, tag="aT", bufs=3)
            nc.vector.tensor_mul(aT[:], araw[:], rb[:])
            aTs[n] = aT
        mlp(NCK - 1)

    # --- stage 1 ------------------------------------------------------------
    def write_out1(n, hT, w2t):
        po = ps.tile([64, 512], F32, tag="mlp", bufs=1)
        nc.tensor.matmul(po[:], w2t[:, 0:64], hT[:, 0:512],
                         start=True, stop=False)
        nc.tensor.matmul(po[:], w2t[:, 64:128], hT[:, 512:1024],
                         start=False, stop=True)
        nc.vector.tensor_scalar_add(outT[:, n * 512:(n + 1) * 512], po[:],
                                    bt[0:64, B2C1:B2C1 + 1])

    def after_mlp1(n):
        exchange(n)
        emit_q(1, qT2, outT, n)

    # bootstrap: first two projection slices before the chunk loop, the rest
    # interleaved into chunk 0's groups.
    proj_unit(0, qT1, kT1, vA1, xT, 0)()
    proj_unit(0, qT1, kT1, vA1, xT, 1)()
    emits1 = {(0, g): [proj_unit(0, qT1, kT1, vA1, xT, g + 2)]
              for g in range(6)}
    stage(0, qT1, kT1, vA1, emits1, write_out1, after_mlp1)

    # --- stage 2 ------------------------------------------------------------
    def write_out2(n, hT, w2t):
        for ss in range(4):
            po2 = ps.tile([128, 64], F32, tag="mlp", bufs=1)
            nc.tensor.matmul(po2[:], hT[:, ss * 128:(ss + 1) * 128],
                             w2t[:, 0:64], start=True, stop=False)
            nc.tensor.matmul(po2[:], hT[:, 512 + ss * 128:512 + (ss + 1) * 128],
                             w2t[:, 64:128], start=False, stop=True)
            fin = sb.tile([128, 64], F32, tag="fin", bufs=3)
            nc.vector.tensor_add(fin[:], po2[:], bt[:, B2R2:B2R2 + 64])
            row0 = n * 512 + ss * 128
            nc.sync.dma_start(out_d[row0:row0 + 128, :], fin[:])

    emits2 = {(0, g): [proj_unit(1, qT2, kT2, vA2, xT2, g, with_q=False)]
              for g in range(8)}
    stage(1, qT2, kT2, vA2, emits2, write_out2, None)


def prep_inputs(x, q, k, v, q1, k1, v1, W1, b1, W2, b2, W11, b11, W22, b22):
    """Returns per-core in_maps for run_bass_kernel_spmd."""
    f = np.float32

    def cast(a):
        return np.ascontiguousarray(np.asarray(a), dtype=f)

    scale = f(0.125)                      # 1/sqrt(QD), folded into wq
    wpack = np.zeros((128, WB), dtype=f)
    bias32 = np.zeros((128, BF32C), dtype=f)
    for sfx, (qq, kk, vv, W1_, b1_, W2_, b2_) in enumerate(
            [(q, k, v, W1, b1, W2, b2), (q1, k1, v1, W11, b11, W22, b22)]):
        c0 = RA * sfx
        wq = cast(qq) * scale
        wpack[0:64, c0 + WQ:c0 + WQ + 64] = wq
        wpack[0:64, c0 + WQ + 64:c0 + WQ + 128] = wq
        wpack[0:64, c0 + WK:c0 + WK + 64] = cast(kk)
        wpack[0:64, c0 + WK + 64:c0 + WK + 128] = cast(kk)
        wpack[0:64, c0 + WV:c0 + WV + 64] = cast(vv)
        wpack[0:64, c0 + W1T:c0 + W1T + HD] = cast(W1_).T
        w2T = cast(W2_).T                                 # [HD, 64]
        for j in range(2):
            wpack[:, W2T0 + sfx * 128 + j * 64:
                  W2T0 + sfx * 128 + (j + 1) * 64] = w2T[j * 128:(j + 1) * 128]
            bias32[:, sfx * 2 + j] = cast(b1_)[j * 128:(j + 1) * 128]
        b2e = cast(b2_) - cast(W2_).sum(axis=1)           # ELU +1 fold
        if sfx == 0:
            bias32[0:64, B2C1] = b2e
        else:
            bias32[:, B2R2:B2R2 + 64] = np.tile(b2e[None, :], (128, 1))

    in_maps = []
    xc = cast(x)
    for c in range(N_CORES):
        b, h = c // 2, c % 2
        xb = xc[b]                      # [S, 64]
        if h == 1:                      # own half first
            xb = np.concatenate([xb[R:], xb[:R]], axis=0)
        in_maps.append({"xT": np.ascontiguousarray(xb.T),
                        "wpack": wpack, "bias32": bias32})
    return in_maps


_NC_CACHE = None


def kernel(**inputs) -> np.ndarray:
    global _NC_CACHE
    if _NC_CACHE is None:
        _NC_CACHE = build_nc()
    nc = _NC_CACHE
    in_maps = prep_inputs(**inputs)
    res = bass_utils.run_bass_kernel_spmd(nc, in_maps,
                                          core_ids=list(range(N_CORES)))
    out = np.empty((B, S, 64), dtype=np.float32)
    for c in range(N_CORES):
        b, h = c // 2, c % 2
        out[b, h * R:(h + 1) * R, :] = res.results[c]["out1"]
    return out
